# revision 39
# baseline (speedup 1.0000x reference)
"""Trainium2 Bass kernel for nn_AttentionLayer_60894046322746.

Full attention layer: fused QKV projection + (elementwise) rotary + softmax
attention with additive bias + output projection.

  B=2, S=2048, HID=1024, NH=16, DH=64, ROT=32, fp32 inputs/outputs.

v2 design (vs v1): NO collectives. 8 cores = 2 batches x 4 query shards;
every core recomputes K and V for its WHOLE batch (4x duplicated PE work,
~60us) instead of AllGathering them (measured 280us of barrier+gather on
this fabric). The bias add inside softmax is done by multiplying
exp(scores) with a host-precomputed exp(bias) (bf16 DVE multiply) instead
of v1's identity-matmul (which was ~109us of PE time). Exp runs on ACT
over 2-bank [128,1024] PSUM tiles to amortize instruction overhead.
Everything is bf16 on the matmul paths (same PE rate as f32r, half the
DMA/SBUF).

The single compiled program is shared by all 8 cores, but each core's Q
phase needs its OWN 512-token chunk of x. Trick: the host rotates the
token-chunk order per core so chunk 0 is always the core's own tokens.
Attention is permutation-invariant over the k axis, so K/V computed in
rotated order stay correct as long as mk (k-rotary map) and exp(bias)
(k-major blocks) are permuted the same way on the host -- they are
per-core inputs anyway. Q/output token order is never permuted.

Per-core compute layout (all matmuls out = lhsT.T @ rhs, contraction on
partitions):
  V[tok, vdim'] = x_tile.T @ Wv        (vdim' has a fused ones column FIRST
                                        per head -> softmax denominator
                                        rides in ctx row 0; custom-DVE
                                        reciprocal_approx_fast requires
                                        partition-base-0 operands on HW)
  K^T[dim, tok] = Wk'.T @ xT           (bias via K=1 ones-row matmul),
                                        then *= mk rotary map (DVE)
  Q^T likewise (own 512 tokens only), *= mq (rotary * 1/sqrt(DH))
  S^T[k, q]     = K_tile.T @ Q_head    (two heads of a pair run
                                        concurrently on PE row-halves)
  E = exp(S^T)                         (ACT, 2 k-tiles per instruction)
  E *= exp(bias)^T                     (DVE bf16, host-precomputed)
  ctx'^T/s      = V'_tile.T @ E        (accumulate over 16 k-tiles;
                                        row 64 = denominator)
  ctx^T = ctx'^T * (1/s)               (DVE; 1/s partition-broadcast)
  out[q, m]     = ctxpair.T @ projW
"""
import os
import sys
import time

for _p in ("/opt/trn_rl_repo", "/root/.axon_site/_ro/trn_rl_repo"):
    if os.path.isdir(_p) and _p not in sys.path:
        sys.path.insert(0, _p)

import numpy as np
import ml_dtypes

from concourse import bass, bacc, tile, mybir
from concourse.bass_utils import run_bass_kernel_spmd

F32 = mybir.dt.float32
BF16 = mybir.dt.bfloat16
FP8 = mybir.dt.float8e4
DR = mybir.MatmulPerfMode.DoubleRow
AF = mybir.ActivationFunctionType
bf16 = ml_dtypes.bfloat16
f8 = ml_dtypes.float8_e4m3
WS = 64.0  # fp8 weight pre-scale (avoids e4m3 subnormals)

B, S, HID = 2, 2048, 1024
DH, NH, ROT = 64, 16, 32
SQ = S // 4            # queries per core
NKT = S // 128         # 16 k-token tiles
NPAIR = NH // 2        # 8 head pairs
N_CORES = 8

_CACHED_NC = None


def _build_nc(dbg=False):
    nc = bacc.Bacc("TRN2", target_bir_lowering=False, debug=False,
                   num_devices=N_CORES)

    # ---- per-core DRAM parameters (host-prepared shards) ----
    xT_d = nc.dram_tensor("xT", [4, 128, 8, 512], BF16, kind="ExternalInput")
    xones_d = nc.dram_tensor("xones", [1, 512], BF16, kind="ExternalInput")
    wqk_d = nc.dram_tensor("wqk", [16, 128, 8, 128], BF16,
                           kind="ExternalInput")
    bcol_d = nc.dram_tensor("bcol", [128, 16], F32, kind="ExternalInput")
    wv_d = nc.dram_tensor("wv", [4, 128, 8, 260], BF16, kind="ExternalInput")
    bv_d = nc.dram_tensor("bv", [1, NH * 65], BF16, kind="ExternalInput")
    mq_d = nc.dram_tensor("mq", [128, SQ], F32, kind="ExternalInput")
    mk_d = nc.dram_tensor("mk", [128, S], F32, kind="ExternalInput")
    expb_d = nc.dram_tensor("expb", [128, NKT, SQ], BF16,
                            kind="ExternalInput")
    projw_d = nc.dram_tensor("projw", [128, 8, HID], BF16,
                             kind="ExternalInput")
    out_d = nc.dram_tensor("out", [SQ, HID], F32, kind="ExternalOutput")

    dbg_d = {}
    if dbg:
        for nm, shp, dt_ in [
            ("dbg_q", [128, SQ], BF16), ("dbg_k", [128, 512], BF16),
            ("dbg_v", [128, NH * 65], BF16), ("dbg_st", [128, 1024], F32),
            ("dbg_e", [128, 1024], BF16), ("dbg_ef", [128, 1024], BF16),
            ("dbg_ctx", [65, SQ], F32), ("dbg_sa", [1, SQ], F32),
            ("dbg_rb", [64, SQ], F32), ("dbg_cp", [128, SQ], BF16),
        ]:
            dbg_d[nm] = nc.dram_tensor(nm, shp, dt_, kind="ExternalOutput")

    with tile.TileContext(nc) as tc:
        _build_body(nc, tc, xT_d, xones_d, wqk_d, bcol_d, wv_d, bv_d,
                    mq_d, mk_d, expb_d, projw_d, out_d, dbg_d)
    nc.compile()
    return nc


def _build_body(nc, tc, xT_d, xones_d, wqk_d, bcol_d, wv_d, bv_d,
                mq_d, mk_d, expb_d, projw_d, out_d, dbg_d=None):
    dbg_d = dbg_d or {}
    with (
        tc.tile_pool(name="persist", bufs=1) as pp,
    ):
        xT_sb = pp.tile([128, 4, 8, 512], BF16, name="xT_sb")
        xones = pp.tile([1, 512], BF16, name="xones")
        bcol_sb = pp.tile([128, 16], F32, name="bcol_sb")
        bv_sb = pp.tile([1, NH * 65], BF16, name="bv_sb")
        mq_sb = pp.tile([128, SQ], F32, name="mq_sb")
        mk_sb = pp.tile([128, S], F32, name="mk_sb")
        expb_sb = pp.tile([128, NKT, SQ], BF16, name="expb_sb")
        projw_sb = pp.tile([128, 8, HID], BF16, name="projw_sb")
        kp_sb = [pp.tile([128, S], BF16, name=f"kp_sb{p}")
                 for p in range(NPAIR)]
        qt_sb = [pp.tile([128, SQ], BF16, name=f"qt_sb{p}")
                 for p in range(NPAIR)]
        vkt_sb = [pp.tile([128, NH, 65], BF16, name=f"vkt_sb{t}")
                  for t in range(NKT)]
        ctxpair = pp.tile([128, NPAIR, SQ], BF16, name="ctxpair")

        # ---- input DMAs ----
        for tch in range(4):
            nc.sync.dma_start(out=xT_sb[:, tch], in_=xT_d[tch])
        nc.sync.dma_start(out=xones[:], in_=xones_d[:])
        nc.sync.dma_start(out=bcol_sb[:], in_=bcol_d[:])
        nc.sync.dma_start(out=bv_sb[:], in_=bv_d[:])

        # ================= V phase (all 2048 tokens) =================
        # c innermost so the stationary x-tile is reused by 4 matmuls
        # (amortizes LDWEIGHTS); 4 PSUM banks live per token tile.
        with (
            tc.tile_pool(name="v_w", bufs=4) as vwp,
            tc.tile_pool(name="v_ps", bufs=5, space="PSUM") as vps,
        ):
            wvcs = []
            for c in range(4):   # 4 chunks of 4 heads = 260 vdims
                wvc = vwp.tile([128, 8, 260], BF16, tag="wv", name="wvc")
                nc.sync.dma_start(out=wvc[:], in_=wv_d[c])
                wvcs.append(wvc)
            for tt in range(NKT):
                tch, j = tt // 4, tt % 4
                pss = [vps.tile([128, 260], F32, tag="vps", name="v_ps")
                       for _ in range(4)]
                for kt in range(8):
                    for c in range(4):
                        nc.tensor.matmul(
                            pss[c][:],
                            xT_sb[:, tch, kt, 128 * j:128 * (j + 1)],
                            wvcs[c][:, kt, :], start=(kt == 0), stop=False)
                for c in range(4):
                    nc.tensor.matmul(
                        pss[c][:], xones[0:1, 0:128],
                        bv_sb[0:1, 260 * c:260 * (c + 1)],
                        start=False, stop=True)
                for c in range(4):
                    nc.vector.tensor_copy(
                        vkt_sb[tt][:, 4 * c:4 * (c + 1), :]
                        .rearrange("p h c -> p (h c)"), pss[c][:])

        # deferred input DMAs (not needed until K/Q/attention) so the
        # V-phase inputs win the DMA queues at kernel start
        nc.sync.dma_start(out=mk_sb[:], in_=mk_d[:])
        nc.sync.dma_start(out=mq_sb[:], in_=mq_d[:])
        nc.sync.dma_start(out=expb_sb[:], in_=expb_d[:])
        nc.sync.dma_start(out=projw_sb[:], in_=projw_d[:])

        # ========== K/Q + attention, pipelined per head-pair ==========
        with (
            tc.tile_pool(name="kq_w", bufs=3) as wp,
            tc.tile_pool(name="kq_ps", bufs=2, space="PSUM") as kqps,
            tc.tile_pool(name="st_ps", bufs=2, space="PSUM") as sps,
            tc.tile_pool(name="ctx_ps", bufs=2, space="PSUM") as cps,
            tc.tile_pool(name="att_e", bufs=3) as ep,
            tc.tile_pool(name="norm", bufs=2) as np_,
        ):
            ADD = mybir.AluOpType.add
            MUL = mybir.AluOpType.mult
            for p in range(NPAIR):
                # --- K dims for pair p: wqk tile 8+p -> kp_sb[p] ---
                # qkv bias is folded into the rotary multiply:
                # kp = (ps + bias_col) * mk  via scalar_tensor_tensor
                wt = wp.tile([128, 8, 128], BF16, tag="wqk", name="wt")
                nc.sync.dma_start(out=wt[:], in_=wqk_d[8 + p])
                for tch in range(4):
                    ps = kqps.tile([128, 512], F32, tag="kq", name="kq_ps")
                    for kt in range(8):
                        nc.tensor.matmul(ps[:], wt[:, kt, :],
                                         xT_sb[:, tch, kt, :],
                                         start=(kt == 0), stop=(kt == 7))
                    nc.vector.scalar_tensor_tensor(
                        out=kp_sb[p][:, 512 * tch:512 * (tch + 1)],
                        in0=ps[:], scalar=bcol_sb[:, 8 + p:9 + p],
                        in1=mk_sb[:, 512 * tch:512 * (tch + 1)],
                        op0=ADD, op1=MUL)

                # --- Q dims for pair p (own tokens only) ---
                wtq = wp.tile([128, 8, 128], BF16, tag="wqk", name="wtq")
                nc.sync.dma_start(out=wtq[:], in_=wqk_d[p])
                # chunk 0 of xT_sb is always the core's own 512 tokens
                psq = kqps.tile([128, 512], F32, tag="kq", name="q_ps")
                for kt in range(8):
                    nc.tensor.matmul(psq[:], wtq[:, kt, :],
                                     xT_sb[:, 0, kt, :],
                                     start=(kt == 0), stop=(kt == 7))
                nc.vector.scalar_tensor_tensor(
                    out=qt_sb[p][:], in0=psq[:],
                    scalar=bcol_sb[:, p:p + 1], in1=mq_sb[:],
                    op0=ADD, op1=MUL)

                if p == 0 and dbg_d:
                    nc.sync.dma_start(out=dbg_d["dbg_q"][:], in_=qt_sb[0][:])
                    nc.sync.dma_start(out=dbg_d["dbg_k"][:],
                                      in_=kp_sb[0][:, 0:512])
                    nc.sync.dma_start(
                        out=dbg_d["dbg_v"][:],
                        in_=vkt_sb[0][:].rearrange("p h c -> p (h c)"))

                # --- attention for pair p ---
                ctx0 = cps.tile([65, SQ], F32, tag="ctx", name="ctx0")
                ctx1 = cps.tile([65, SQ], F32, tag="ctx", name="ctx1")
                # one [128, 2(head), 512] scores tile per k-tile: the two
                # head matmuls target disjoint PE row groups (0-63 /
                # 64-127 via base_partition) and share one ACT consumer,
                # so they can run concurrently when issued adjacently
                for kt in range(NKT):
                    kc = slice(128 * kt, 128 * (kt + 1))
                    st = sps.tile([128, 2, 512], F32, tag="st", name="st")
                    nc.tensor.matmul(st[:, 0, :], kp_sb[p][0:64, kc],
                                     qt_sb[p][0:64, :],
                                     start=True, stop=True)
                    nc.tensor.matmul(st[:, 1, :], kp_sb[p][64:128, kc],
                                     qt_sb[p][64:128, :],
                                     start=True, stop=True)
                    e = ep.tile([128, 1024], BF16, tag="e", name="e")
                    nc.scalar.activation(
                        e[:], st[:].rearrange("p a b -> p (a b)"), AF.Exp)
                    # exp(bias) multiply: same bias for both heads; split
                    # the two head-halves across DVE and GpSimd
                    ef = ep.tile([128, 1024], BF16, tag="ef", name="ef")
                    nc.vector.tensor_mul(ef[:, 0:512], e[:, 0:512],
                                         expb_sb[:, kt, :])
                    nc.gpsimd.tensor_mul(ef[:, 512:1024], e[:, 512:1024],
                                         expb_sb[:, kt, :])
                    if p == 0 and kt == 0 and dbg_d:
                        sdump = ep.tile([128, 2, 512], F32, tag="sd",
                                        name="sdump")
                        nc.vector.tensor_copy(sdump[:], st[:])
                        nc.sync.dma_start(
                            out=dbg_d["dbg_st"][:],
                            in_=sdump[:].rearrange("p a b -> p (a b)"))
                        nc.sync.dma_start(out=dbg_d["dbg_e"][:], in_=e[:])
                        nc.sync.dma_start(out=dbg_d["dbg_ef"][:], in_=ef[:])
                    nc.tensor.matmul(ctx0[:], vkt_sb[kt][:, 2 * p, :],
                                     ef[:, 0:512],
                                     start=(kt == 0), stop=(kt == 15))
                    nc.tensor.matmul(ctx1[:], vkt_sb[kt][:, 2 * p + 1, :],
                                     ef[:, 512:1024],
                                     start=(kt == 0), stop=(kt == 15))

                # --- normalize: ctx^T = ctx'[0:64] * bcast(1/ctx'[64]) ---
                # plain tensor_copy moves the denominator row from
                # partition 64 to 0 (cross-base is fine for plain DVE ops
                # but NOT for custom-DVE ones, and APs must be 32-aligned)
                for hi, ctx in enumerate((ctx0, ctx1)):
                    sden = np_.tile([1, SQ], F32, tag="sd", name="sden")
                    nc.vector.tensor_copy(sden[0:1, :], ctx[64:65, :])
                    sa = np_.tile([1, SQ], F32, tag="sa", name="sa")
                    nc.vector.reciprocal_approx_fast(sa[0:1, :],
                                                     sden[0:1, :])
                    rb = np_.tile([64, SQ], F32, tag="rb", name="rb")
                    nc.gpsimd.partition_broadcast(rb[:], sa[0:1, :])
                    if p == 0 and hi == 0 and dbg_d:
                        cdump = np_.tile([65, SQ], F32, tag="cd",
                                         name="cdump")
                        nc.vector.tensor_copy(cdump[:], ctx[:])
                        nc.sync.dma_start(out=dbg_d["dbg_ctx"][:],
                                          in_=cdump[:])
                        nc.sync.dma_start(out=dbg_d["dbg_sa"][:],
                                          in_=sa[:])
                        nc.sync.dma_start(out=dbg_d["dbg_rb"][:],
                                          in_=rb[:])
                    nc.vector.tensor_mul(
                        ctxpair[64 * hi:64 * (hi + 1), p, :],
                        ctx[0:64, :], rb[:])
                if p == 0 and dbg_d:
                    nc.sync.dma_start(out=dbg_d["dbg_cp"][:],
                                      in_=ctxpair[:, 0, :])

        # ================= projection phase =================
        with (
            tc.tile_pool(name="proj_ps", bufs=2, space="PSUM") as pps,
            tc.tile_pool(name="proj_out", bufs=3) as pop,
        ):
            for qt in range(4):
                for n in range(2):
                    ps = pps.tile([128, 512], F32, tag="pps", name="proj_ps")
                    for pr in range(8):
                        nc.tensor.matmul(
                            ps[:],
                            ctxpair[:, pr, 128 * qt:128 * (qt + 1)],
                            projw_sb[:, pr, 512 * n:512 * (n + 1)],
                            start=(pr == 0), stop=(pr == 7))
                    ot = pop.tile([128, 512], F32, tag="ot", name="ot")
                    nc.vector.tensor_copy(ot[:], ps[:])
                    nc.sync.dma_start(
                        out=out_d[128 * qt:128 * (qt + 1),
                                  512 * n:512 * (n + 1)],
                        in_=ot[:])


# ---------------- host-side prep ----------------

def _make_rotary_map(sinusoids):
    sin = np.asarray(sinusoids[0], np.float32).T  # [ROT, S]
    cos = np.asarray(sinusoids[1], np.float32).T
    M = np.ones((DH, S), np.float32)
    sign = np.where(np.arange(ROT) % 2 == 0, -1.0, 1.0).astype(np.float32)
    M[:ROT] = cos + sign[:, None] * sin
    return M


def _host_prep(x, sinusoids, attention_bias, qkv_kernel, qkv_bias,
               proj_kernel):
    x = np.asarray(x, np.float32)
    sinusoids = np.asarray(sinusoids, np.float32)
    attention_bias = np.asarray(attention_bias, np.float32)
    qkv_kernel = np.asarray(qkv_kernel, np.float32)
    qkv_bias = np.asarray(qkv_bias, np.float32)
    proj_kernel = np.asarray(proj_kernel, np.float32)

    M = _make_rotary_map(sinusoids)
    scale = np.float32(1.0 / np.sqrt(DH))

    # wqk [HID, 2048]: cols 0-1023 Q dims, 1024-2047 K dims
    wqk = qkv_kernel[:, :32, :].reshape(HID, 2048)
    # -> [16 dim-tiles, 128 p, 8 kt, 128 c]
    wqk_t = np.ascontiguousarray(
        wqk.reshape(8, 128, 16, 128).transpose(2, 1, 0, 3)).astype(bf16)
    # per-dim-tile bias columns [128, 16] (fp32, folded into the rotary
    # multiply on DVE)
    bcol = np.ascontiguousarray(
        qkv_bias[:32].reshape(16, 128).T).astype(np.float32)

    # V weights with per-head trailing ones column: [HID, NH, 65] -> chunks
    wv = np.zeros((HID, NH, 65), np.float32)
    wv[:, :, :64] = qkv_kernel[:, 32:, :]
    wv = wv.reshape(HID, NH * 65)
    wv_t = np.ascontiguousarray(
        wv.reshape(8, 128, 4, 260).transpose(2, 1, 0, 3)).astype(bf16)
    bv = np.zeros((NH, 65), np.float32)
    bv[:, :64] = qkv_bias[32:]
    bv[:, 64] = 1.0
    bv = bv.reshape(1, NH * 65).astype(bf16)

    projw = proj_kernel.reshape(HID, HID)
    projw_t = np.ascontiguousarray(
        projw.reshape(8, 128, 1024).transpose(1, 0, 2)).astype(bf16)

    mk_full = np.tile(M, (2, 1))                                 # [128, S]
    xones = np.ones((1, 512), np.float32).astype(bf16)

    in_maps = []
    for i in range(N_CORES):
        b, r = i // 4, i % 4
        qs = slice(SQ * r, SQ * (r + 1))
        # per-core k-token chunk rotation: chunk 0 = own tokens
        perm = [(r + c) % 4 for c in range(4)]
        ktg = [perm[t // 4] * 4 + t % 4 for t in range(NKT)]
        xT = np.ascontiguousarray(x[b].T)                        # [HID, S]
        xT_t = np.ascontiguousarray(
            xT.reshape(8, 128, 4, 512).transpose(2, 1, 0, 3)[perm]
        ).astype(bf16)
        mk = np.ascontiguousarray(
            mk_full.reshape(128, 4, 512)[:, perm, :].reshape(128, S))
        mq = np.ascontiguousarray(np.tile(M[:, qs] * scale, (2, 1)))
        # exp(bias)^T -> [128 kpart, NKT, SQ], k-blocks in permuted order
        eb = np.exp(attention_bias[b, 0, qs, :]).T               # [S, SQ]
        eb_t = np.ascontiguousarray(
            eb.reshape(NKT, 128, SQ)[ktg].transpose(1, 0, 2)).astype(bf16)
        in_maps.append({
            "xT": xT_t, "xones": xones, "wqk": wqk_t, "bcol": bcol,
            "wv": wv_t, "bv": bv, "mq": mq, "mk": mk,
            "expb": eb_t, "projw": projw_t,
        })
    return in_maps


def kernel(x, sinusoids, attention_bias, qkv_kernel, qkv_bias, proj_kernel,
           **_ignored):
    global _CACHED_NC
    if _CACHED_NC is None:
        _CACHED_NC = _build_nc()
    nc = _CACHED_NC

    in_maps = _host_prep(x, sinusoids, attention_bias, qkv_kernel,
                         qkv_bias, proj_kernel)
    trace = bool(os.environ.get("BASS_TRACE"))
    res = run_bass_kernel_spmd(nc, in_maps, core_ids=list(range(N_CORES)),
                               trace=trace)
    if res.exec_time_ns is not None:
        print(f"HW exec time: {res.exec_time_ns} ns")

    out = np.zeros((B, S, HID), np.float32)
    for i in range(N_CORES):
        b, r = i // 4, i % 4
        out[b, SQ * r:SQ * (r + 1), :] = res.results[i]["out"]
    return out


if __name__ == "__main__":
    rng = np.random.default_rng(0)
    ins = dict(
        x=rng.standard_normal((B, S, HID)).astype(np.float32),
        sinusoids=rng.uniform(-1, 1, (2, S, ROT)).astype(np.float32),
        attention_bias=(rng.standard_normal((B, 1, S, S)) * 0.1).astype(
            np.float32),
        qkv_kernel=(rng.standard_normal((HID, 48, DH)) * 0.0124).astype(
            np.float32),
        qkv_bias=np.zeros((48, DH), np.float32),
        proj_kernel=(rng.standard_normal((NH, DH, HID)) * 0.0124).astype(
            np.float32),
    )
    t0 = time.time()
    out = kernel(**ins)
    print(f"kernel() wall: {time.time()-t0:.1f}s out shape {out.shape}")


# revision 42
# speedup vs baseline: 1.3614x; 1.3614x over previous
"""Trainium2 Bass kernel for nn_AttentionLayer_60894046322746.

Full attention layer: fused QKV projection + (elementwise) rotary + softmax
attention with additive bias + output projection.

  B=2, S=2048, HID=1024, NH=16, DH=64, ROT=32, fp32 inputs/outputs.

v2 design (vs v1): NO collectives. 8 cores = 2 batches x 4 query shards;
every core recomputes K and V for its WHOLE batch (4x duplicated PE work,
~60us) instead of AllGathering them (measured 280us of barrier+gather on
this fabric). The bias add inside softmax is done by multiplying
exp(scores) with a host-precomputed exp(bias) (bf16 DVE multiply) instead
of v1's identity-matmul (which was ~109us of PE time). Exp runs on ACT
over 2-bank [128,1024] PSUM tiles to amortize instruction overhead.
Everything is bf16 on the matmul paths (same PE rate as f32r, half the
DMA/SBUF).

The single compiled program is shared by all 8 cores, but each core's Q
phase needs its OWN 512-token chunk of x. Trick: the host rotates the
token-chunk order per core so chunk 0 is always the core's own tokens.
Attention is permutation-invariant over the k axis, so K/V computed in
rotated order stay correct as long as mk (k-rotary map) and exp(bias)
(k-major blocks) are permuted the same way on the host -- they are
per-core inputs anyway. Q/output token order is never permuted.

Per-core compute layout (all matmuls out = lhsT.T @ rhs, contraction on
partitions):
  V[tok, vdim'] = x_tile.T @ Wv        (vdim' has a fused ones column FIRST
                                        per head -> softmax denominator
                                        rides in ctx row 0; custom-DVE
                                        reciprocal_approx_fast requires
                                        partition-base-0 operands on HW)
  K^T[dim, tok] = Wk'.T @ xT           (bias via K=1 ones-row matmul),
                                        then *= mk rotary map (DVE)
  Q^T likewise (own 512 tokens only), *= mq (rotary * 1/sqrt(DH))
  S^T[k, q]     = K_tile.T @ Q_head    (two heads of a pair run
                                        concurrently on PE row-halves)
  E = exp(S^T)                         (ACT, 2 k-tiles per instruction)
  E *= exp(bias)^T                     (DVE bf16, host-precomputed)
  ctx'^T/s      = V'_tile.T @ E        (accumulate over 16 k-tiles;
                                        row 64 = denominator)
  ctx^T = ctx'^T * (1/s)               (DVE; 1/s partition-broadcast)
  out[q, m]     = ctxpair.T @ projW
"""
import os
import sys
import time

for _p in ("/opt/trn_rl_repo", "/root/.axon_site/_ro/trn_rl_repo"):
    if os.path.isdir(_p) and _p not in sys.path:
        sys.path.insert(0, _p)

import numpy as np
import ml_dtypes

from concourse import bass, bacc, tile, mybir
from concourse.bass_utils import run_bass_kernel_spmd

F32 = mybir.dt.float32
BF16 = mybir.dt.bfloat16
FP8 = mybir.dt.float8e4
DR = mybir.MatmulPerfMode.DoubleRow
AF = mybir.ActivationFunctionType
bf16 = ml_dtypes.bfloat16
f8 = ml_dtypes.float8_e4m3
WS = 64.0  # fp8 weight pre-scale (avoids e4m3 subnormals)

B, S, HID = 2, 2048, 1024
DH, NH, ROT = 64, 16, 32
SQ = S // 4            # queries per core
NKT = S // 128         # 16 k-token tiles
NPAIR = NH // 2        # 8 head pairs
N_CORES = 8

_CACHED_NC = None


def _build_nc(dbg=False):
    nc = bacc.Bacc("TRN2", target_bir_lowering=False, debug=False,
                   num_devices=N_CORES)

    # ---- per-core DRAM parameters (host-prepared shards) ----
    xT_d = nc.dram_tensor("xT", [4, 128, 8, 512], BF16, kind="ExternalInput")
    xones_d = nc.dram_tensor("xones", [1, 512], BF16, kind="ExternalInput")
    wqk_d = nc.dram_tensor("wqk", [16, 128, 8, 128], BF16,
                           kind="ExternalInput")
    bcol_d = nc.dram_tensor("bcol", [128, 16], F32, kind="ExternalInput")
    wv_d = nc.dram_tensor("wv", [4, 128, 8, 260], BF16, kind="ExternalInput")
    bv_d = nc.dram_tensor("bv", [1, NH * 65], BF16, kind="ExternalInput")
    mq_d = nc.dram_tensor("mq", [128, SQ], F32, kind="ExternalInput")
    mk_d = nc.dram_tensor("mk", [128, S], F32, kind="ExternalInput")
    expb_d = nc.dram_tensor("expb", [128, NKT, 2 * SQ], BF16,
                            kind="ExternalInput")
    projw_d = nc.dram_tensor("projw", [128, 8, HID], BF16,
                             kind="ExternalInput")
    out_d = nc.dram_tensor("out", [SQ, HID], F32, kind="ExternalOutput")

    dbg_d = {}
    if dbg:
        for nm, shp, dt_ in [
            ("dbg_q", [128, SQ], BF16), ("dbg_k", [128, 512], BF16),
            ("dbg_v", [128, NH * 65], BF16), ("dbg_st", [128, 1024], F32),
            ("dbg_e", [128, 1024], BF16), ("dbg_ef", [128, 1024], BF16),
            ("dbg_ctx", [65, SQ], F32), ("dbg_sa", [1, SQ], F32),
            ("dbg_rb", [64, SQ], F32), ("dbg_cp", [128, SQ], BF16),
        ]:
            dbg_d[nm] = nc.dram_tensor(nm, shp, dt_, kind="ExternalOutput")

    with tile.TileContext(nc) as tc:
        _build_body(nc, tc, xT_d, xones_d, wqk_d, bcol_d, wv_d, bv_d,
                    mq_d, mk_d, expb_d, projw_d, out_d, dbg_d)
    nc.compile()
    return nc


def _build_body(nc, tc, xT_d, xones_d, wqk_d, bcol_d, wv_d, bv_d,
                mq_d, mk_d, expb_d, projw_d, out_d, dbg_d=None):
    dbg_d = dbg_d or {}
    with (
        tc.tile_pool(name="persist", bufs=1) as pp,
    ):
        xT_sb = pp.tile([128, 4, 8, 512], BF16, name="xT_sb")
        xones = pp.tile([1, 512], BF16, name="xones")
        bcol_sb = pp.tile([128, 16], F32, name="bcol_sb")
        bv_sb = pp.tile([1, NH * 65], BF16, name="bv_sb")
        mq_sb = pp.tile([128, SQ], F32, name="mq_sb")
        mk_sb = pp.tile([128, S], F32, name="mk_sb")
        # exp(bias), duplicated along the free axis so one DVE multiply
        # covers both head-halves of an e tile
        expb_sb = pp.tile([128, NKT, 2 * SQ], BF16, name="expb_sb")
        projw_sb = pp.tile([128, 8, HID], BF16, name="projw_sb")
        kp_sb = [pp.tile([128, S], BF16, name=f"kp_sb{p}")
                 for p in range(NPAIR)]
        qt_sb = [pp.tile([128, SQ], BF16, name=f"qt_sb{p}")
                 for p in range(NPAIR)]
        vkt_sb = [pp.tile([128, NH, 65], BF16, name=f"vkt_sb{t}")
                  for t in range(NKT)]
        ctxpair = pp.tile([128, NPAIR, SQ], BF16, name="ctxpair")

        # ---- input DMAs ----
        for tch in range(4):
            nc.sync.dma_start(out=xT_sb[:, tch], in_=xT_d[tch])
        nc.sync.dma_start(out=xones[:], in_=xones_d[:])
        nc.sync.dma_start(out=bcol_sb[:], in_=bcol_d[:])
        nc.sync.dma_start(out=bv_sb[:], in_=bv_d[:])

        # ================= V phase (all 2048 tokens) =================
        # c innermost so the stationary x-tile is reused by 4 matmuls
        # (amortizes LDWEIGHTS); 4 PSUM banks live per token tile.
        with (
            tc.tile_pool(name="v_w", bufs=4) as vwp,
            tc.tile_pool(name="v_ps", bufs=5, space="PSUM") as vps,
        ):
            wvcs = []
            for c in range(4):   # 4 chunks of 4 heads = 260 vdims
                wvc = vwp.tile([128, 8, 260], BF16, tag="wv", name="wvc")
                nc.sync.dma_start(out=wvc[:], in_=wv_d[c])
                wvcs.append(wvc)
            for tt in range(NKT):
                tch, j = tt // 4, tt % 4
                pss = [vps.tile([128, 260], F32, tag="vps", name="v_ps")
                       for _ in range(4)]
                for kt in range(8):
                    for c in range(4):
                        nc.tensor.matmul(
                            pss[c][:],
                            xT_sb[:, tch, kt, 128 * j:128 * (j + 1)],
                            wvcs[c][:, kt, :], start=(kt == 0), stop=False)
                for c in range(4):
                    nc.tensor.matmul(
                        pss[c][:], xones[0:1, 0:128],
                        bv_sb[0:1, 260 * c:260 * (c + 1)],
                        start=False, stop=True)
                for c in range(4):
                    nc.vector.tensor_copy(
                        vkt_sb[tt][:, 4 * c:4 * (c + 1), :]
                        .rearrange("p h c -> p (h c)"), pss[c][:])

        # deferred input DMAs (not needed until K/Q/attention) so the
        # V-phase inputs win the DMA queues at kernel start
        nc.sync.dma_start(out=mk_sb[:], in_=mk_d[:])
        nc.sync.dma_start(out=mq_sb[:], in_=mq_d[:])
        nc.sync.dma_start(out=expb_sb[:], in_=expb_d[:])
        nc.sync.dma_start(out=projw_sb[:], in_=projw_d[:])

        # ========== K/Q + attention, pipelined per head-pair ==========
        with (
            tc.tile_pool(name="kq_w", bufs=3) as wp,
            tc.tile_pool(name="kq_ps", bufs=2, space="PSUM") as kqps,
            tc.tile_pool(name="st_ps", bufs=2, space="PSUM") as sps,
            tc.tile_pool(name="ctx_ps", bufs=2, space="PSUM") as cps,
            tc.tile_pool(name="att_e", bufs=3) as ep,
            tc.tile_pool(name="norm", bufs=2) as np_,
        ):
            ADD = mybir.AluOpType.add
            MUL = mybir.AluOpType.mult
            for p in range(NPAIR):
                # --- K dims for pair p: wqk tile 8+p -> kp_sb[p] ---
                # qkv bias is folded into the rotary multiply:
                # kp = (ps + bias_col) * mk  via scalar_tensor_tensor
                wt = wp.tile([128, 8, 128], BF16, tag="wqk", name="wt")
                nc.sync.dma_start(out=wt[:], in_=wqk_d[8 + p])
                for tch in range(4):
                    ps = kqps.tile([128, 512], F32, tag="kq", name="kq_ps")
                    for kt in range(8):
                        nc.tensor.matmul(ps[:], wt[:, kt, :],
                                         xT_sb[:, tch, kt, :],
                                         start=(kt == 0), stop=(kt == 7))
                    nc.vector.scalar_tensor_tensor(
                        out=kp_sb[p][:, 512 * tch:512 * (tch + 1)],
                        in0=ps[:], scalar=bcol_sb[:, 8 + p:9 + p],
                        in1=mk_sb[:, 512 * tch:512 * (tch + 1)],
                        op0=ADD, op1=MUL)

                # --- Q dims for pair p (own tokens only) ---
                wtq = wp.tile([128, 8, 128], BF16, tag="wqk", name="wtq")
                nc.sync.dma_start(out=wtq[:], in_=wqk_d[p])
                # chunk 0 of xT_sb is always the core's own 512 tokens
                psq = kqps.tile([128, 512], F32, tag="kq", name="q_ps")
                for kt in range(8):
                    nc.tensor.matmul(psq[:], wtq[:, kt, :],
                                     xT_sb[:, 0, kt, :],
                                     start=(kt == 0), stop=(kt == 7))
                nc.vector.scalar_tensor_tensor(
                    out=qt_sb[p][:], in0=psq[:],
                    scalar=bcol_sb[:, p:p + 1], in1=mq_sb[:],
                    op0=ADD, op1=MUL)

                if p == 0 and dbg_d:
                    nc.sync.dma_start(out=dbg_d["dbg_q"][:], in_=qt_sb[0][:])
                    nc.sync.dma_start(out=dbg_d["dbg_k"][:],
                                      in_=kp_sb[0][:, 0:512])
                    nc.sync.dma_start(
                        out=dbg_d["dbg_v"][:],
                        in_=vkt_sb[0][:].rearrange("p h c -> p (h c)"))

                # --- attention for pair p ---
                ctx0 = cps.tile([65, SQ], F32, tag="ctx", name="ctx0")
                ctx1 = cps.tile([65, SQ], F32, tag="ctx", name="ctx1")
                # one [128, 2(head), 512] scores tile per k-tile: the two
                # head matmuls target disjoint PE row groups (0-63 /
                # 64-127 via base_partition) and share one ACT consumer,
                # so they can run concurrently when issued adjacently
                for kt in range(NKT):
                    kc = slice(128 * kt, 128 * (kt + 1))
                    st = sps.tile([128, 2, 512], F32, tag="st", name="st")
                    nc.tensor.matmul(st[:, 0, :], kp_sb[p][0:64, kc],
                                     qt_sb[p][0:64, :],
                                     start=True, stop=True)
                    nc.tensor.matmul(st[:, 1, :], kp_sb[p][64:128, kc],
                                     qt_sb[p][64:128, :],
                                     start=True, stop=True)
                    e = ep.tile([128, 1024], BF16, tag="e", name="e")
                    nc.scalar.activation(
                        e[:], st[:].rearrange("p a b -> p (a b)"), AF.Exp)
                    # exp(bias) multiply (same bias for both head-halves)
                    ef = ep.tile([128, 1024], BF16, tag="ef", name="ef")
                    nc.vector.tensor_mul(ef[:], e[:], expb_sb[:, kt, :])
                    if p == 0 and kt == 0 and dbg_d:
                        sdump = ep.tile([128, 2, 512], F32, tag="sd",
                                        name="sdump")
                        nc.vector.tensor_copy(sdump[:], st[:])
                        nc.sync.dma_start(
                            out=dbg_d["dbg_st"][:],
                            in_=sdump[:].rearrange("p a b -> p (a b)"))
                        nc.sync.dma_start(out=dbg_d["dbg_e"][:], in_=e[:])
                        nc.sync.dma_start(out=dbg_d["dbg_ef"][:], in_=ef[:])
                    nc.tensor.matmul(ctx0[:], vkt_sb[kt][:, 2 * p, :],
                                     ef[:, 0:512],
                                     start=(kt == 0), stop=(kt == 15))
                    nc.tensor.matmul(ctx1[:], vkt_sb[kt][:, 2 * p + 1, :],
                                     ef[:, 512:1024],
                                     start=(kt == 0), stop=(kt == 15))

                # --- normalize: ctx^T = ctx'[0:64] * bcast(1/ctx'[64]) ---
                # plain tensor_copy moves the denominator row from
                # partition 64 to 0 (cross-base is fine for plain DVE ops
                # but NOT for custom-DVE ones, and APs must be 32-aligned)
                for hi, ctx in enumerate((ctx0, ctx1)):
                    sden = np_.tile([1, SQ], F32, tag="sd", name="sden")
                    nc.vector.tensor_copy(sden[0:1, :], ctx[64:65, :])
                    sa = np_.tile([1, SQ], F32, tag="sa", name="sa")
                    nc.vector.reciprocal_approx_fast(sa[0:1, :],
                                                     sden[0:1, :])
                    rb = np_.tile([64, SQ], F32, tag="rb", name="rb")
                    nc.gpsimd.partition_broadcast(rb[:], sa[0:1, :])
                    if p == 0 and hi == 0 and dbg_d:
                        cdump = np_.tile([65, SQ], F32, tag="cd",
                                         name="cdump")
                        nc.vector.tensor_copy(cdump[:], ctx[:])
                        nc.sync.dma_start(out=dbg_d["dbg_ctx"][:],
                                          in_=cdump[:])
                        nc.sync.dma_start(out=dbg_d["dbg_sa"][:],
                                          in_=sa[:])
                        nc.sync.dma_start(out=dbg_d["dbg_rb"][:],
                                          in_=rb[:])
                    nc.vector.tensor_mul(
                        ctxpair[64 * hi:64 * (hi + 1), p, :],
                        ctx[0:64, :], rb[:])
                if p == 0 and dbg_d:
                    nc.sync.dma_start(out=dbg_d["dbg_cp"][:],
                                      in_=ctxpair[:, 0, :])

        # ================= projection phase =================
        with (
            tc.tile_pool(name="proj_ps", bufs=2, space="PSUM") as pps,
            tc.tile_pool(name="proj_out", bufs=3) as pop,
        ):
            for qt in range(4):
                for n in range(2):
                    ps = pps.tile([128, 512], F32, tag="pps", name="proj_ps")
                    for pr in range(8):
                        nc.tensor.matmul(
                            ps[:],
                            ctxpair[:, pr, 128 * qt:128 * (qt + 1)],
                            projw_sb[:, pr, 512 * n:512 * (n + 1)],
                            start=(pr == 0), stop=(pr == 7))
                    ot = pop.tile([128, 512], F32, tag="ot", name="ot")
                    nc.vector.tensor_copy(ot[:], ps[:])
                    nc.sync.dma_start(
                        out=out_d[128 * qt:128 * (qt + 1),
                                  512 * n:512 * (n + 1)],
                        in_=ot[:])


# ---------------- host-side prep ----------------

def _make_rotary_map(sinusoids):
    sin = np.asarray(sinusoids[0], np.float32).T  # [ROT, S]
    cos = np.asarray(sinusoids[1], np.float32).T
    M = np.ones((DH, S), np.float32)
    sign = np.where(np.arange(ROT) % 2 == 0, -1.0, 1.0).astype(np.float32)
    M[:ROT] = cos + sign[:, None] * sin
    return M


def _host_prep(x, sinusoids, attention_bias, qkv_kernel, qkv_bias,
               proj_kernel):
    x = np.asarray(x, np.float32)
    sinusoids = np.asarray(sinusoids, np.float32)
    attention_bias = np.asarray(attention_bias, np.float32)
    qkv_kernel = np.asarray(qkv_kernel, np.float32)
    qkv_bias = np.asarray(qkv_bias, np.float32)
    proj_kernel = np.asarray(proj_kernel, np.float32)

    M = _make_rotary_map(sinusoids)
    scale = np.float32(1.0 / np.sqrt(DH))

    # wqk [HID, 2048]: cols 0-1023 Q dims, 1024-2047 K dims
    wqk = qkv_kernel[:, :32, :].reshape(HID, 2048)
    # -> [16 dim-tiles, 128 p, 8 kt, 128 c]
    wqk_t = np.ascontiguousarray(
        wqk.reshape(8, 128, 16, 128).transpose(2, 1, 0, 3)).astype(bf16)
    # per-dim-tile bias columns [128, 16] (fp32, folded into the rotary
    # multiply on DVE)
    bcol = np.ascontiguousarray(
        qkv_bias[:32].reshape(16, 128).T).astype(np.float32)

    # V weights with per-head trailing ones column: [HID, NH, 65] -> chunks
    wv = np.zeros((HID, NH, 65), np.float32)
    wv[:, :, :64] = qkv_kernel[:, 32:, :]
    wv = wv.reshape(HID, NH * 65)
    wv_t = np.ascontiguousarray(
        wv.reshape(8, 128, 4, 260).transpose(2, 1, 0, 3)).astype(bf16)
    bv = np.zeros((NH, 65), np.float32)
    bv[:, :64] = qkv_bias[32:]
    bv[:, 64] = 1.0
    bv = bv.reshape(1, NH * 65).astype(bf16)

    projw = proj_kernel.reshape(HID, HID)
    projw_t = np.ascontiguousarray(
        projw.reshape(8, 128, 1024).transpose(1, 0, 2)).astype(bf16)

    mk_full = np.tile(M, (2, 1))                                 # [128, S]
    xones = np.ones((1, 512), np.float32).astype(bf16)

    in_maps = []
    for i in range(N_CORES):
        b, r = i // 4, i % 4
        qs = slice(SQ * r, SQ * (r + 1))
        # per-core k-token chunk rotation: chunk 0 = own tokens
        perm = [(r + c) % 4 for c in range(4)]
        ktg = [perm[t // 4] * 4 + t % 4 for t in range(NKT)]
        xT = np.ascontiguousarray(x[b].T)                        # [HID, S]
        xT_t = np.ascontiguousarray(
            xT.reshape(8, 128, 4, 512).transpose(2, 1, 0, 3)[perm]
        ).astype(bf16)
        mk = np.ascontiguousarray(
            mk_full.reshape(128, 4, 512)[:, perm, :].reshape(128, S))
        mq = np.ascontiguousarray(np.tile(M[:, qs] * scale, (2, 1)))
        # exp(bias)^T -> [128 kpart, NKT, SQ], k-blocks in permuted order
        eb = np.exp(attention_bias[b, 0, qs, :]).T               # [S, SQ]
        eb_t = eb.reshape(NKT, 128, SQ)[ktg].transpose(1, 0, 2)
        eb_t = np.ascontiguousarray(
            np.concatenate([eb_t, eb_t], axis=2)).astype(bf16)
        in_maps.append({
            "xT": xT_t, "xones": xones, "wqk": wqk_t, "bcol": bcol,
            "wv": wv_t, "bv": bv, "mq": mq, "mk": mk,
            "expb": eb_t, "projw": projw_t,
        })
    return in_maps


def kernel(x, sinusoids, attention_bias, qkv_kernel, qkv_bias, proj_kernel,
           **_ignored):
    global _CACHED_NC
    if _CACHED_NC is None:
        _CACHED_NC = _build_nc()
    nc = _CACHED_NC

    in_maps = _host_prep(x, sinusoids, attention_bias, qkv_kernel,
                         qkv_bias, proj_kernel)
    trace = bool(os.environ.get("BASS_TRACE"))
    res = run_bass_kernel_spmd(nc, in_maps, core_ids=list(range(N_CORES)),
                               trace=trace)
    if res.exec_time_ns is not None:
        print(f"HW exec time: {res.exec_time_ns} ns")

    out = np.zeros((B, S, HID), np.float32)
    for i in range(N_CORES):
        b, r = i // 4, i % 4
        out[b, SQ * r:SQ * (r + 1), :] = res.results[i]["out"]
    return out


if __name__ == "__main__":
    rng = np.random.default_rng(0)
    ins = dict(
        x=rng.standard_normal((B, S, HID)).astype(np.float32),
        sinusoids=rng.uniform(-1, 1, (2, S, ROT)).astype(np.float32),
        attention_bias=(rng.standard_normal((B, 1, S, S)) * 0.1).astype(
            np.float32),
        qkv_kernel=(rng.standard_normal((HID, 48, DH)) * 0.0124).astype(
            np.float32),
        qkv_bias=np.zeros((48, DH), np.float32),
        proj_kernel=(rng.standard_normal((NH, DH, HID)) * 0.0124).astype(
            np.float32),
    )
    t0 = time.time()
    out = kernel(**ins)
    print(f"kernel() wall: {time.time()-t0:.1f}s out shape {out.shape}")
